# revision 48
# baseline (speedup 1.0000x reference)
"""GNN message-passing + pooling kernel for 8 Trainium2 NeuronCores.

Strategy (per the sharding hint):
  - Host: sort edges by dst, partition the 50k nodes into 8 contiguous
    ranges of 6250; each core gets the edges targeting its node range
    (disjoint scatter -> no cross-core reduction needed).
  - Host gathers x[dst], x[src], edge_attr into a transposed bf16
    [320, E_pad] tensor per core (edges grouped into 481-node scatter
    windows, padded to a uniform chunk count so the device program is
    identical across cores).
  - Device (per core): 4-layer message MLP in transposed-activation
    layout processed in 2048-edge blocks (4x512 supertiles).  Each
    weight chunk is kept stationary on the PE array for 4 consecutive
    matmuls (amortizes LDWEIGHTS, which otherwise serializes ~100ns per
    matmul).  Layer 4 is computed weight-stationary into a feature-major
    [msg_dim, edges] PSUM tile, bias fused into the PSUM->SBUF copy on
    the scalar engine, then DMA-transposed (xbar) into edge-major
    [128, 128] chunks for the scatter.  Scatter-add via one-hot matmuls
    (one-hot built on DVE with iota + is_equal against per-edge local
    dst), deferred by one block so the transposes are off the critical
    path.  Node MLP over the core's 6250 nodes with the same blocked
    structure, per-graph sum-pooling accumulated in a single PSUM bank.
    Output: [32, 128] partial per-graph sums.
  - Host: sum the 8 partials, divide by per-graph node counts, apply the
    final [128, 16] linear.
"""

import sys

if "/opt/trn_rl_repo" not in sys.path:
    sys.path.insert(0, "/opt/trn_rl_repo")

import numpy as np
import ml_dtypes

BF16 = ml_dtypes.bfloat16

# Problem dims
N_NODES = 50000
N_EDGES = 800000
NF = 128          # node feature dim
EF = 64           # edge feature dim
MSGD = 128        # message dim
HID = 300         # MLP hidden
G = 32            # graphs
NCORES = 8

# Tiling config
NPC = N_NODES // NCORES   # 6250 nodes per core
NW = 241                  # nodes per scatter window
W = 26                    # windows per core (26*241 = 6266 >= 6250)
ST = 512                  # edge supertile (free dim per matmul)
BST = 4                   # supertiles per block (weight-stationary reuse)
NP2 = 6656                # padded nodes per core for node MLP (13*512)
NT = NP2 // ST            # node supertiles
NCHK = NP2 // 128         # node chunks for pooling
HIDP = 384                # HID zero-padded to full 128-row K chunks
KINP = 384                # 2*NF+EF zero-padded likewise

TRACE = False             # set True from test harness to profile core 0
LAST_EXEC_NS = None

_BUILD_CACHE = {}


def _chunks(total, step=128):
    return [(o, min(step, total - o)) for o in range(0, total, step)]


def _build_nc(C):
    """Build the (single) SPMD Bass program. C = 128-edge chunks per window
    (multiple of 16 so each window is a whole number of 2048-edge blocks)."""
    import concourse.bacc as bacc
    import concourse.tile as tile
    from concourse import mybir
    from contextlib import ExitStack

    f32 = mybir.dt.float32
    bf16 = mybir.dt.bfloat16
    AF = mybir.ActivationFunctionType
    OP = mybir.AluOpType

    fp8 = mybir.dt.float8e4
    E_pad = W * C * 128
    NCHUNKS = W * C
    NBLK = NCHUNKS // 16      # 2048-edge blocks

    nc = bacc.Bacc("TRN2", target_bir_lowering=False, debug=False,
                   num_devices=NCORES)

    # --- DRAM I/O ---
    # L1 input: x[dst]/x[src] features as fp8 DoubleRow pair, edge_attr
    # (+zero pad) as bf16 remainder
    d_min_dr = nc.dram_tensor("min_dr", [128, 2, E_pad], fp8,
                              kind="ExternalInput")
    d_min_rem = nc.dram_tensor("min_rem", [128, E_pad], bf16,
                               kind="ExternalInput")
    d_dstloc = nc.dram_tensor("dstloc", [128, NCHUNKS], f32,
                              kind="ExternalInput")
    d_xT = nc.dram_tensor("xT", [NF, NP2], bf16, kind="ExternalInput")
    d_pmat = nc.dram_tensor("pmat", [128, NCHK * G], bf16,
                            kind="ExternalInput")

    d_mW4 = nc.dram_tensor("mW4", [HIDP, MSGD], bf16, kind="ExternalInput")
    d_nW1 = nc.dram_tensor("nW1", [NF + MSGD, HIDP], bf16,
                           kind="ExternalInput")
    d_nW4 = nc.dram_tensor("nW4", [HIDP, NF], bf16, kind="ExternalInput")
    # L2/L3 weights: fp8 DoubleRow pair (K rows 0..255, x8 scale) + bf16
    # remainder (K rows 256..383, x8 scale)
    d_wdr = {}
    d_wrem = {}
    for nm in ("mW1", "mW2", "mW3", "nW2", "nW3"):
        d_wdr[nm] = nc.dram_tensor(f"{nm}dr", [128, 2 * HIDP], fp8,
                                   kind="ExternalInput")
        d_wrem[nm] = nc.dram_tensor(f"{nm}rem", [128, HIDP], bf16,
                                    kind="ExternalInput")
    d_mb = [nc.dram_tensor(f"mb{i}", [HIDP, 1], f32, kind="ExternalInput")
            for i in range(1, 4)]
    d_mb4c = nc.dram_tensor("mb4c", [MSGD, 1], f32, kind="ExternalInput")
    d_nb = [nc.dram_tensor(f"nb{i}", [HIDP, 1], f32, kind="ExternalInput")
            for i in range(1, 4)]
    d_nb4c = nc.dram_tensor("nb4c", [NF, 1], f32, kind="ExternalInput")
    d_out = nc.dram_tensor("partial", [G, NF], f32, kind="ExternalOutput")

    HCH = _chunks(HIDP)         # [(0,128),(128,128),(256,128)]
    KIN = _chunks(KINP)         # [(0,128),(128,128),(256,128)]

    with tile.TileContext(nc) as tc, ExitStack() as ctx:
        wpool = ctx.enter_context(tc.tile_pool(name="w", bufs=1))
        apool = ctx.enter_context(tc.tile_pool(name="agg", bufs=1))
        inpool = ctx.enter_context(tc.tile_pool(name="in", bufs=3))
        hpool = ctx.enter_context(tc.tile_pool(name="h", bufs=2))
        mtpool = ctx.enter_context(tc.tile_pool(name="mt", bufs=2))
        spool = ctx.enter_context(tc.tile_pool(name="s", bufs=20))
        mm_psum = ctx.enter_context(
            tc.tile_pool(name="mmp", bufs=7, space="PSUM"))
        acc_psum = ctx.enter_context(
            tc.tile_pool(name="accp", bufs=1, space="PSUM"))

        def load_w(dram, K, N, dt, name):
            tiles = []
            for i, (k0, kk) in enumerate(_chunks(K)):
                t = wpool.tile([kk, N], dt, tag=f"{name}{i}")
                nc.sync.dma_start(t[:, :], dram[k0:k0 + kk, :])
                tiles.append(t)
            return tiles

        def load_block(blk):
            base = blk * (BST * ST)
            ind = inpool.tile([128, 2, BST * ST], fp8, tag="ind")
            nc.sync.dma_start(ind[:, :, :],
                              d_min_dr[:, :, base:base + BST * ST])
            inr = inpool.tile([128, BST * ST], bf16, tag="inr")
            nc.sync.dma_start(inr[:, :],
                              d_min_rem[:, base:base + BST * ST])
            return ind, inr

        # first block's inputs + L1 weights first so the PE can start early;
        # the rest of the (large) resident loads follow on the same queue
        pending_in = load_block(0)
        wdr = {}
        wrem = {}

        def load_drpair(nm):
            t3 = wpool.tile([128, 2, HIDP], fp8, tag=f"{nm}dr")
            nc.sync.dma_start(t3[:, :, :], d_wdr[nm][:, :])
            wdr[nm] = t3
            tr = wpool.tile([128, HIDP], bf16, tag=f"{nm}rem")
            nc.sync.dma_start(tr[:, :], d_wrem[nm][:, :])
            wrem[nm] = tr

        # edge-phase weights in first-use order, node-phase tensors last
        load_drpair("mW1")
        load_drpair("mW2")
        load_drpair("mW3")
        mb = [load_w(d_mb[i], HIDP, 1, f32, f"mb{i + 1}") for i in range(3)]
        mW4 = load_w(d_mW4, HIDP, MSGD, bf16, "mW4")
        mb4c = wpool.tile([MSGD, 1], f32, tag="mb4c")
        nc.sync.dma_start(mb4c[:, :], d_mb4c[:, :])
        dstloc = wpool.tile([128, NCHUNKS], f32, tag="dstloc")
        nc.sync.dma_start(dstloc[:, :], d_dstloc[:, :])
        nW1 = load_w(d_nW1, NF + MSGD, HIDP, bf16, "nW1")
        load_drpair("nW2")
        load_drpair("nW3")
        nb = [load_w(d_nb[i], HIDP, 1, f32, f"nb{i + 1}") for i in range(3)]
        nW4 = load_w(d_nW4, HIDP, NF, bf16, "nW4")
        nb4c = wpool.tile([NF, 1], f32, tag="nb4c")
        nc.sync.dma_start(nb4c[:, :], d_nb4c[:, :])
        xT = wpool.tile([NF, NP2], bf16, tag="xT")
        nc.sync.dma_start(xT[:, :], d_xT[:, :])
        pmat = wpool.tile([128, NCHK * G], bf16, tag="pmat")
        nc.sync.dma_start(pmat[:, :], d_pmat[:, :])

        iota = wpool.tile([128, NW], f32, tag="iota")
        nc.gpsimd.iota(iota[:, :], pattern=[[1, NW]], base=0,
                       channel_multiplier=0,
                       allow_small_or_imprecise_dtypes=True)

        aggrT = apool.tile([NF, NP2], bf16, tag="aggrT")
        # scatter windows cover cols [0, W*NW); zero the tail
        nc.gpsimd.memset(aggrT[:, W * NW:NP2], 0.0)

        DRM = mybir.MatmulPerfMode.DoubleRow

        def mlp_front(l1, w2d, w2r, w3d, w3r, biases, gs):
            """Layers 1-3, weight-stationary over gs supertiles; fp8
            DoubleRow (K 0..255) + bf16 remainder for L2/L3 (and edge L1).
            l1 = ("dr", w1d, w1r, get_ind, get_inr) or ("bf16", ksrc_list).
            Returns the layer-3 h chunk tiles."""
            # ---- L1 -> h1 fp8 dr-pair + bf16 remainder ----
            hd1 = hpool.tile([128, 2, BST * ST], fp8, tag="hd0")
            hr1 = hpool.tile([128, BST * ST], bf16, tag="hr0")
            for m, (m0, mm) in enumerate(HCH):
                pss = []
                for g in range(gs):
                    p = mm_psum.tile([128, ST], f32, tag="mmp")
                    pss.append(p)
                if l1[0] == "dr":
                    _, w1d, w1r, get_ind, get_inr = l1
                    for g in range(gs):
                        nc.tensor.matmul(pss[g][:mm, :], w1d[:, :, m0:m0 + mm],
                                         get_ind(g), start=True, stop=False,
                                         perf_mode=DRM)
                    for g in range(gs):
                        nc.tensor.matmul(pss[g][:mm, :], w1r[:, m0:m0 + mm],
                                         get_inr(g), start=False, stop=True)
                else:
                    ksrc = l1[1]
                    for k, (get_rhs, kk, wt) in enumerate(ksrc):
                        lhs = wt[:, m0:m0 + mm]
                        for g in range(gs):
                            nc.tensor.matmul(pss[g][:mm, :], lhs, get_rhs(g),
                                             start=(k == 0),
                                             stop=(k == len(ksrc) - 1))
                for g in range(gs):
                    dst = (hd1[:, m, g * ST:(g + 1) * ST] if m < 2
                           else hr1[:, g * ST:(g + 1) * ST])
                    nc.vector.tensor_scalar(
                        dst, pss[g][:mm, :], biases[0][m][:mm, :], 0.0,
                        op0=OP.add, op1=OP.max)

            # ---- L2 (fp8 DR) -> h2 fp8 dr-pair + bf16 remainder ----
            hd2 = hpool.tile([128, 2, BST * ST], fp8, tag="hd1")
            hr2 = hpool.tile([128, BST * ST], bf16, tag="hr1")
            for m, (m0, mm) in enumerate(HCH):
                pss = []
                for g in range(gs):
                    p = mm_psum.tile([128, ST], f32, tag="mmp")
                    pss.append(p)
                for g in range(gs):
                    nc.tensor.matmul(pss[g][:mm, :], w2d[:, :, m0:m0 + mm],
                                     hd1[:, :, g * ST:(g + 1) * ST],
                                     start=True, stop=False, perf_mode=DRM)
                for g in range(gs):
                    nc.tensor.matmul(pss[g][:mm, :], w2r[:, m0:m0 + mm],
                                     hr1[:, g * ST:(g + 1) * ST],
                                     start=False, stop=True)
                for g in range(gs):
                    if m < 2:
                        nc.scalar.activation(
                            hd2[:, m, g * ST:(g + 1) * ST], pss[g][:mm, :],
                            AF.Relu, bias=biases[1][m][:mm, :])
                    else:
                        nc.vector.tensor_scalar(
                            hr2[:, g * ST:(g + 1) * ST], pss[g][:mm, :],
                            biases[1][m][:mm, :], 0.0, op0=OP.add, op1=OP.max)

            # ---- L3 (fp8 DR) -> h3 bf16 chunks ----
            cur = []
            for m, (m0, mm) in enumerate(HCH):
                pss = []
                for g in range(gs):
                    p = mm_psum.tile([128, ST], f32, tag="mmp")
                    pss.append(p)
                for g in range(gs):
                    nc.tensor.matmul(pss[g][:mm, :], w3d[:, :, m0:m0 + mm],
                                     hd2[:, :, g * ST:(g + 1) * ST],
                                     start=True, stop=False, perf_mode=DRM)
                for g in range(gs):
                    nc.tensor.matmul(pss[g][:mm, :], w3r[:, m0:m0 + mm],
                                     hr2[:, g * ST:(g + 1) * ST],
                                     start=False, stop=True)
                ht = hpool.tile([128, BST * ST], bf16, tag=f"h2_{m}")
                for g in range(gs):
                    nc.scalar.activation(ht[:mm, g * ST:(g + 1) * ST],
                                         pss[g][:mm, :], AF.Relu,
                                         bias=biases[2][m][:mm, :])
                cur.append(ht)
            return cur

        # ---- edge phase ----
        acc_state = {}

        def build_sts(blk):
            sts = []
            for i in range(16):
                cidx = blk * 16 + i
                st = spool.tile([128, NW], bf16, tag="st")
                nc.gpsimd.tensor_scalar(
                    st[:, :], iota[:, :], dstloc[:, cidx:cidx + 1], None,
                    op0=OP.is_equal)
                sts.append(st)
            return sts

        def emit_scatter(blk, msgts, sts):
            for i, mt in enumerate(msgts):
                cidx = blk * 16 + i
                w = cidx // C
                cw = cidx % C
                if cw == 0:
                    at = acc_psum.tile([128, NW], f32, tag="acc")
                    acc_state["t"] = at
                nc.tensor.matmul(acc_state["t"][:, :], mt[:, :], sts[i][:, :],
                                 start=(cw == 0), stop=(cw == C - 1),
                                 skip_group_check=True)
                if cw == C - 1:
                    nc.scalar.activation(
                        aggrT[:, w * NW:(w + 1) * NW], acc_state["t"][:, :],
                        AF.Copy)

        prev_msgts = None
        prev_sts = None
        prev_blk = None
        for blk in range(NBLK):
            ind, inr = pending_in
            if blk + 1 < NBLK:
                pending_in = load_block(blk + 1)
            if prev_msgts is not None:
                prev_sts = build_sts(prev_blk)

            l1 = ("dr", wdr["mW1"], wrem["mW1"],
                  lambda g, t=ind: t[:, :, g * ST:(g + 1) * ST],
                  lambda g, t=inr: t[:, g * ST:(g + 1) * ST])

            h3 = mlp_front(l1, wdr["mW2"], wrem["mW2"],
                           wdr["mW3"], wrem["mW3"], mb, BST)

            # L4 weight-stationary: out [MSGD, ST] feature-major
            psl4 = []
            for g in range(BST):
                p4 = mm_psum.tile([128, ST], f32, tag="mmp")
                psl4.append(p4)
            for k, (k0, kk) in enumerate(HCH):
                for g in range(BST):
                    nc.tensor.matmul(psl4[g][:, :], mW4[k][:kk, :],
                                     h3[k][:kk, g * ST:(g + 1) * ST],
                                     start=(k == 0), stop=(k == 2))
            msgts = []
            for g in range(BST):
                mT = mtpool.tile([128, ST], bf16, tag=f"msgT{g}")
                nc.scalar.activation(mT[:, :], psl4[g][:, :], AF.Identity,
                                     bias=mb4c[:, :])
                mt4 = mtpool.tile([128, 4, 128], bf16, tag=f"msgt{g}")
                nc.sync.dma_start_transpose(mt4[:, :, :], mT[:, :])
                for e in range(4):
                    msgts.append(mt4[:, e, :])

            if prev_msgts is not None:
                emit_scatter(prev_blk, prev_msgts, prev_sts)
            prev_msgts, prev_blk = msgts, blk
        prev_sts = build_sts(prev_blk)
        emit_scatter(prev_blk, prev_msgts, prev_sts)

        # ---- node phase ----
        # L4 weight-stationary into feature-major [NF, nodes] PSUM (bias
        # fused in the copy), DMA-transposed to node-major chunks for the
        # pooling matmuls, which are deferred by one block.
        pool_acc = acc_psum.tile([128, NW], f32, tag="acc")

        def flush_pool(t0, gs, nts):
            for g in range(gs):
                for e in range(4):
                    tch = (t0 + g) * 4 + e
                    nc.tensor.matmul(pool_acc[:G, :NF],
                                     pmat[:, tch * G:(tch + 1) * G],
                                     nts[g][:, e, :],
                                     start=(tch == 0), stop=(tch == NCHK - 1),
                                     skip_group_check=True)

        pend_nt = None
        for t0 in range(0, NT, BST):
            gs = min(BST, NT - t0)

            nksrc = ("bf16", [
                ((lambda g, s=xT, t0=t0:
                  s[:, (t0 + g) * ST:(t0 + g + 1) * ST]), NF, nW1[0]),
                ((lambda g, s=aggrT, t0=t0:
                  s[:, (t0 + g) * ST:(t0 + g + 1) * ST]), MSGD, nW1[1])])

            h3n = mlp_front(nksrc, wdr["nW2"], wrem["nW2"],
                            wdr["nW3"], wrem["nW3"], nb, gs)

            psn = []
            for g in range(gs):
                p4 = mm_psum.tile([128, ST], f32, tag="mmp")
                psn.append(p4)
            for k, (k0, kk) in enumerate(HCH):
                for g in range(gs):
                    nc.tensor.matmul(psn[g][:, :], nW4[k][:kk, :],
                                     h3n[k][:kk, g * ST:(g + 1) * ST],
                                     start=(k == 0), stop=(k == 2))
            nts = []
            for g in range(gs):
                nT = mtpool.tile([128, ST], bf16, tag=f"msgT{g}")
                nc.scalar.activation(nT[:, :], psn[g][:, :], AF.Identity,
                                     bias=nb4c[:, :])
                nt4 = mtpool.tile([128, 4, 128], bf16, tag=f"msgt{g}")
                nc.sync.dma_start_transpose(nt4[:, :, :], nT[:, :])
                nts.append(nt4)
            if pend_nt is not None:
                flush_pool(*pend_nt)
            pend_nt = (t0, gs, nts)
        flush_pool(*pend_nt)

        pooled = apool.tile([G, NF], f32, tag="pooled")
        nc.vector.tensor_copy(pooled[:, :], pool_acc[:G, :NF])
        nc.sync.dma_start(d_out[:, :], pooled[:, :])

    nc.compile()
    return nc


def _prep_inputs(x, edge_index, edge_attr, batch, weights, C):
    """Host-side shard/gather/pad. Returns per-core in_maps."""
    E_pad = W * C * 128
    src = np.asarray(edge_index[0], np.int64)
    dst = np.asarray(edge_index[1], np.int64)

    order = np.argsort(dst, kind="stable")
    dsts = dst[order]
    srcs = src[order]

    xT = np.ascontiguousarray(np.asarray(x, np.float32).astype(BF16).T)
    eaT = np.ascontiguousarray(np.asarray(edge_attr, np.float32).astype(BF16).T)
    batch = np.asarray(batch, np.int64)

    bounds = np.searchsorted(dsts, np.arange(0, N_NODES + 1, NPC))

    def pad2(a, r, c):
        out = np.zeros((r, c), a.dtype)
        out[:a.shape[0], :a.shape[1]] = a
        return out

    FP8 = ml_dtypes.float8_e4m3fn
    wcommon = {}
    wcommon["mW4"] = pad2(weights["mW4"].astype(BF16), HIDP, MSGD)
    wcommon["nW1"] = pad2(weights["nW1"].astype(BF16), NF + MSGD, HIDP)
    wcommon["nW4"] = pad2(weights["nW4"].astype(BF16), HIDP, NF)
    for nm in ("mW1", "mW2", "mW3", "nW2", "nW3"):
        kr = KINP if nm == "mW1" else HIDP
        wp = pad2(weights[nm].astype(np.float32), kr, HIDP)
        dr = wp[:256].reshape(2, 128, HIDP).transpose(1, 0, 2)
        wcommon[f"{nm}dr"] = np.ascontiguousarray(
            dr.reshape(128, 2 * HIDP).astype(FP8))
        wcommon[f"{nm}rem"] = np.ascontiguousarray(wp[256:kr].astype(BF16))
    for i in range(1, 4):
        wcommon[f"mb{i}"] = pad2(
            weights[f"mb{i}"].reshape(HID, 1).astype(np.float32), HIDP, 1)
        wcommon[f"nb{i}"] = pad2(
            weights[f"nb{i}"].reshape(HID, 1).astype(np.float32), HIDP, 1)
    wcommon["mb4c"] = np.ascontiguousarray(
        weights["mb4"].reshape(MSGD, 1).astype(np.float32))
    wcommon["nb4c"] = np.ascontiguousarray(
        weights["nb4"].reshape(NF, 1).astype(np.float32))

    garange = np.arange(G)
    in_maps = []
    for k in range(NCORES):
        sl = slice(int(bounds[k]), int(bounds[k + 1]))
        eidx = order[sl]
        dloc = dsts[sl] - k * NPC
        srck = srcs[sl]
        win = dloc // NW
        cnt = np.bincount(win, minlength=W)

        starts = np.repeat(np.arange(W) * C * 128, cnt)
        within = np.arange(len(dloc)) - np.repeat(np.cumsum(cnt) - cnt, cnt)
        pos = starts + within

        min_dr = np.zeros((128, 2, E_pad), FP8)
        min_dr[:, 0, pos] = xT[:, k * NPC + dloc].astype(FP8)
        min_dr[:, 1, pos] = xT[:, srck].astype(FP8)
        min_rem = np.zeros((128, E_pad), BF16)
        min_rem[:EF, pos] = eaT[:, eidx]

        dl = np.full(E_pad, -1.0, np.float32)
        dl[pos] = (dloc - win * NW).astype(np.float32)
        dstloc = np.ascontiguousarray(dl.reshape(E_pad // 128, 128).T)

        xTn = np.zeros((NF, NP2), BF16)
        xTn[:, :NPC] = xT[:, k * NPC:(k + 1) * NPC]

        bl = np.full(NP2, -1, np.int64)
        bl[:NPC] = batch[k * NPC:(k + 1) * NPC]
        P = (bl[:, None] == garange[None, :]).astype(BF16)
        pmat = np.ascontiguousarray(
            P.reshape(NCHK, 128, G).transpose(1, 0, 2).reshape(128, NCHK * G))

        in_map = dict(wcommon)
        in_map.update(min_dr=min_dr, min_rem=min_rem, dstloc=dstloc, xT=xTn, pmat=pmat)
        in_maps.append(in_map)
    return in_maps


def kernel(**inputs):
    global LAST_EXEC_NS
    from concourse.bass_utils import run_bass_kernel_spmd

    x = np.asarray(inputs["x"], np.float32)
    edge_index = np.asarray(inputs["edge_index"])
    edge_attr = np.asarray(inputs["edge_attr"], np.float32)
    batch = np.asarray(inputs["batch"])

    # chunk count per window from the actual data (uniform across cores)
    dst = np.asarray(edge_index[1], np.int64)
    dloc_all = dst % NPC
    core_all = dst // NPC
    win_all = dloc_all // NW
    cnt = np.bincount(core_all * W + win_all, minlength=NCORES * W)
    C = int(np.ceil(cnt.max() / 128.0))
    C = max(C, 8)
    while (W * C) % 16 != 0:
        C += 1

    key = C
    if key not in _BUILD_CACHE:
        _BUILD_CACHE[key] = _build_nc(C)
    nc = _BUILD_CACHE[key]

    in_maps = _prep_inputs(x, edge_index, edge_attr, batch, inputs, C)

    res = run_bass_kernel_spmd(nc, in_maps, list(range(NCORES)), trace=TRACE)
    LAST_EXEC_NS = res.exec_time_ns

    total = np.zeros((G, NF), np.float64)
    for r in res.results:
        total += np.asarray(r["partial"], np.float64)

    counts = np.bincount(np.asarray(batch, np.int64), minlength=G)
    pooled = (total / np.maximum(counts, 1)[:, None]).astype(np.float32)
    out = pooled @ np.asarray(inputs["linW"], np.float32) + np.asarray(
        inputs["linb"], np.float32)
    return out.astype(np.float32)


# revision 51
# speedup vs baseline: 2.7574x; 2.7574x over previous
"""GNN message-passing + pooling kernel for 8 Trainium2 NeuronCores.

Strategy (per the sharding hint):
  - Host: sort edges by dst, partition the 50k nodes into 8 contiguous
    ranges of 6250; each core gets the edges targeting its node range
    (disjoint scatter -> no cross-core reduction needed).
  - Host gathers x[dst], x[src], edge_attr into a transposed bf16
    [320, E_pad] tensor per core (edges grouped into 481-node scatter
    windows, padded to a uniform chunk count so the device program is
    identical across cores).
  - Device (per core): 4-layer message MLP in transposed-activation
    layout processed in 2048-edge blocks (4x512 supertiles).  Each
    weight chunk is kept stationary on the PE array for 4 consecutive
    matmuls (amortizes LDWEIGHTS, which otherwise serializes ~100ns per
    matmul).  Layer 4 is computed weight-stationary into a feature-major
    [msg_dim, edges] PSUM tile, bias fused into the PSUM->SBUF copy on
    the scalar engine, then DMA-transposed (xbar) into edge-major
    [128, 128] chunks for the scatter.  Scatter-add via one-hot matmuls
    (one-hot built on DVE with iota + is_equal against per-edge local
    dst), deferred by one block so the transposes are off the critical
    path.  Node MLP over the core's 6250 nodes with the same blocked
    structure, per-graph sum-pooling accumulated in a single PSUM bank.
    Output: [32, 128] partial per-graph sums.
  - Host: sum the 8 partials, divide by per-graph node counts, apply the
    final [128, 16] linear.
"""

import sys

if "/opt/trn_rl_repo" not in sys.path:
    sys.path.insert(0, "/opt/trn_rl_repo")

import numpy as np
import ml_dtypes

BF16 = ml_dtypes.bfloat16

# Problem dims
N_NODES = 50000
N_EDGES = 800000
NF = 128          # node feature dim
EF = 64           # edge feature dim
MSGD = 128        # message dim
HID = 300         # MLP hidden
G = 32            # graphs
NCORES = 8

# Tiling config
NPC = N_NODES // NCORES   # 6250 nodes per core
NW = 241                  # nodes per scatter window
W = 26                    # windows per core (26*241 = 6266 >= 6250)
ST = 512                  # edge supertile (free dim per matmul)
BST = 4                   # supertiles per block (weight-stationary reuse)
NP2 = 6656                # padded nodes per core for node MLP (13*512)
NT = NP2 // ST            # node supertiles
NCHK = NP2 // 128         # node chunks for pooling
HIDP = 384                # HID zero-padded to full 128-row K chunks
KINP = 384                # 2*NF+EF zero-padded likewise

TRACE = False             # set True from test harness to profile core 0
LAST_EXEC_NS = None

_BUILD_CACHE = {}


def _chunks(total, step=128):
    return [(o, min(step, total - o)) for o in range(0, total, step)]


def _build_nc(C):
    """Build the (single) SPMD Bass program. C = 128-edge chunks per window
    (multiple of 16 so each window is a whole number of 2048-edge blocks)."""
    import concourse.bacc as bacc
    import concourse.tile as tile
    from concourse import mybir
    from contextlib import ExitStack

    f32 = mybir.dt.float32
    bf16 = mybir.dt.bfloat16
    AF = mybir.ActivationFunctionType
    OP = mybir.AluOpType

    fp8 = mybir.dt.float8e4
    E_pad = W * C * 128
    NCHUNKS = W * C
    NBLK = NCHUNKS // 16      # 2048-edge blocks

    nc = bacc.Bacc("TRN2", target_bir_lowering=False, debug=False,
                   num_devices=NCORES)

    # --- DRAM I/O ---
    # L1 input: x[dst]/x[src] features as fp8 DoubleRow pair, edge_attr
    # (+zero pad) as bf16 remainder
    d_min_dr = nc.dram_tensor("min_dr", [128, 2, E_pad], fp8,
                              kind="ExternalInput")
    d_min_rem = nc.dram_tensor("min_rem", [128, E_pad], bf16,
                               kind="ExternalInput")
    d_dstloc = nc.dram_tensor("dstloc", [128, NCHUNKS], f32,
                              kind="ExternalInput")
    d_xT = nc.dram_tensor("xT", [NF, NP2], bf16, kind="ExternalInput")
    d_pmat = nc.dram_tensor("pmat", [128, NCHK * G], bf16,
                            kind="ExternalInput")

    d_mW4 = nc.dram_tensor("mW4", [HIDP, MSGD], bf16, kind="ExternalInput")
    d_nW1 = nc.dram_tensor("nW1", [NF + MSGD, HIDP], bf16,
                           kind="ExternalInput")
    d_nW4 = nc.dram_tensor("nW4", [HIDP, NF], bf16, kind="ExternalInput")
    # L2/L3 weights: fp8 DoubleRow pair (K rows 0..255, x8 scale) + bf16
    # remainder (K rows 256..383, x8 scale)
    d_wdr = {}
    d_wrem = {}
    for nm in ("mW1", "mW2", "mW3", "nW2", "nW3"):
        d_wdr[nm] = nc.dram_tensor(f"{nm}dr", [128, 2 * HIDP], fp8,
                                   kind="ExternalInput")
        d_wrem[nm] = nc.dram_tensor(f"{nm}rem", [128, HIDP], bf16,
                                    kind="ExternalInput")
    d_mb = [nc.dram_tensor(f"mb{i}", [HIDP, 1], f32, kind="ExternalInput")
            for i in range(1, 4)]
    d_mb4c = nc.dram_tensor("mb4c", [MSGD, 1], f32, kind="ExternalInput")
    d_nb = [nc.dram_tensor(f"nb{i}", [HIDP, 1], f32, kind="ExternalInput")
            for i in range(1, 4)]
    d_nb4c = nc.dram_tensor("nb4c", [NF, 1], f32, kind="ExternalInput")
    d_out = nc.dram_tensor("partial", [G, NF], f32, kind="ExternalOutput")

    HCH = _chunks(HIDP)         # [(0,128),(128,128),(256,128)]
    KIN = _chunks(KINP)         # [(0,128),(128,128),(256,128)]

    with tile.TileContext(nc) as tc, ExitStack() as ctx:
        wpool = ctx.enter_context(tc.tile_pool(name="w", bufs=1))
        apool = ctx.enter_context(tc.tile_pool(name="agg", bufs=1))
        inpool = ctx.enter_context(tc.tile_pool(name="in", bufs=3))
        hpool = ctx.enter_context(tc.tile_pool(name="h", bufs=2))
        mtpool = ctx.enter_context(tc.tile_pool(name="mt", bufs=2))
        spool = ctx.enter_context(tc.tile_pool(name="s", bufs=20))
        mm_psum = ctx.enter_context(
            tc.tile_pool(name="mmp", bufs=7, space="PSUM"))
        acc_psum = ctx.enter_context(
            tc.tile_pool(name="accp", bufs=1, space="PSUM"))

        def load_w(dram, K, N, dt, name):
            tiles = []
            for i, (k0, kk) in enumerate(_chunks(K)):
                t = wpool.tile([kk, N], dt, tag=f"{name}{i}")
                nc.sync.dma_start(t[:, :], dram[k0:k0 + kk, :])
                tiles.append(t)
            return tiles

        def load_block(blk):
            base = blk * (BST * ST)
            ind = inpool.tile([128, 2, BST * ST], fp8, tag="ind")
            nc.sync.dma_start(ind[:, :, :],
                              d_min_dr[:, :, base:base + BST * ST])
            inr = inpool.tile([128, BST * ST], bf16, tag="inr")
            nc.sync.dma_start(inr[:, :],
                              d_min_rem[:, base:base + BST * ST])
            return ind, inr

        # first block's inputs + L1 weights first so the PE can start early;
        # the rest of the (large) resident loads follow on the same queue
        pending_in = load_block(0)
        wdr = {}
        wrem = {}

        def load_drpair(nm):
            t3 = wpool.tile([128, 2, HIDP], fp8, tag=f"{nm}dr")
            nc.sync.dma_start(t3[:, :, :], d_wdr[nm][:, :])
            wdr[nm] = t3
            tr = wpool.tile([128, HIDP], bf16, tag=f"{nm}rem")
            nc.sync.dma_start(tr[:, :], d_wrem[nm][:, :])
            wrem[nm] = tr

        # edge-phase weights in first-use order, node-phase tensors last
        load_drpair("mW1")
        load_drpair("mW2")
        load_drpair("mW3")
        mb = [load_w(d_mb[i], HIDP, 1, f32, f"mb{i + 1}") for i in range(3)]
        mW4 = load_w(d_mW4, HIDP, MSGD, bf16, "mW4")
        mb4c = wpool.tile([MSGD, 1], f32, tag="mb4c")
        nc.sync.dma_start(mb4c[:, :], d_mb4c[:, :])
        dstloc = wpool.tile([128, NCHUNKS], f32, tag="dstloc")
        nc.sync.dma_start(dstloc[:, :], d_dstloc[:, :])
        nW1 = load_w(d_nW1, NF + MSGD, HIDP, bf16, "nW1")
        load_drpair("nW2")
        load_drpair("nW3")
        nb = [load_w(d_nb[i], HIDP, 1, f32, f"nb{i + 1}") for i in range(3)]
        nW4 = load_w(d_nW4, HIDP, NF, bf16, "nW4")
        nb4c = wpool.tile([NF, 1], f32, tag="nb4c")
        nc.sync.dma_start(nb4c[:, :], d_nb4c[:, :])
        xT = wpool.tile([NF, NP2], bf16, tag="xT")
        nc.sync.dma_start(xT[:, :], d_xT[:, :])
        pmat = wpool.tile([128, NCHK * G], bf16, tag="pmat")
        nc.sync.dma_start(pmat[:, :], d_pmat[:, :])

        iota = wpool.tile([128, NW], bf16, tag="iota")
        nc.gpsimd.iota(iota[:, :], pattern=[[1, NW]], base=0,
                       channel_multiplier=0,
                       allow_small_or_imprecise_dtypes=True)

        aggrT = apool.tile([NF, NP2], bf16, tag="aggrT")
        # scatter windows cover cols [0, W*NW); zero the tail
        nc.gpsimd.memset(aggrT[:, W * NW:NP2], 0.0)

        DRM = mybir.MatmulPerfMode.DoubleRow

        def mlp_front(l1, w2d, w2r, w3d, w3r, biases, gs):
            """Layers 1-3, weight-stationary over gs supertiles; fp8
            DoubleRow (K 0..255) + bf16 remainder for L2/L3 (and edge L1).
            l1 = ("dr", w1d, w1r, get_ind, get_inr) or ("bf16", ksrc_list).
            Returns the layer-3 h chunk tiles."""
            # ---- L1 -> h1 fp8 dr-pair + bf16 remainder ----
            hd1 = hpool.tile([128, 2, BST * ST], fp8, tag="hd0")
            hr1 = hpool.tile([128, BST * ST], bf16, tag="hr0")
            for m, (m0, mm) in enumerate(HCH):
                pss = []
                for g in range(gs):
                    p = mm_psum.tile([128, ST], f32, tag="mmp")
                    pss.append(p)
                if l1[0] == "dr":
                    _, w1d, w1r, get_ind, get_inr = l1
                    for g in range(gs):
                        nc.tensor.matmul(pss[g][:mm, :], w1d[:, :, m0:m0 + mm],
                                         get_ind(g), start=True, stop=False,
                                         perf_mode=DRM)
                    for g in range(gs):
                        nc.tensor.matmul(pss[g][:mm, :], w1r[:, m0:m0 + mm],
                                         get_inr(g), start=False, stop=True)
                else:
                    ksrc = l1[1]
                    for k, (get_rhs, kk, wt) in enumerate(ksrc):
                        lhs = wt[:, m0:m0 + mm]
                        for g in range(gs):
                            nc.tensor.matmul(pss[g][:mm, :], lhs, get_rhs(g),
                                             start=(k == 0),
                                             stop=(k == len(ksrc) - 1))
                for g in range(gs):
                    dst = (hd1[:, m, g * ST:(g + 1) * ST] if m < 2
                           else hr1[:, g * ST:(g + 1) * ST])
                    nc.vector.tensor_scalar(
                        dst, pss[g][:mm, :], biases[0][m][:mm, :], 0.0,
                        op0=OP.add, op1=OP.max)

            # ---- L2 (fp8 DR) -> h2 fp8 dr-pair + bf16 remainder ----
            hd2 = hpool.tile([128, 2, BST * ST], fp8, tag="hd1")
            hr2 = hpool.tile([128, BST * ST], bf16, tag="hr1")
            for m, (m0, mm) in enumerate(HCH):
                pss = []
                for g in range(gs):
                    p = mm_psum.tile([128, ST], f32, tag="mmp")
                    pss.append(p)
                for g in range(gs):
                    nc.tensor.matmul(pss[g][:mm, :], w2d[:, :, m0:m0 + mm],
                                     hd1[:, :, g * ST:(g + 1) * ST],
                                     start=True, stop=False, perf_mode=DRM)
                for g in range(gs):
                    nc.tensor.matmul(pss[g][:mm, :], w2r[:, m0:m0 + mm],
                                     hr1[:, g * ST:(g + 1) * ST],
                                     start=False, stop=True)
                for g in range(gs):
                    if m < 2:
                        nc.scalar.activation(
                            hd2[:, m, g * ST:(g + 1) * ST], pss[g][:mm, :],
                            AF.Relu, bias=biases[1][m][:mm, :])
                    else:
                        nc.vector.tensor_scalar(
                            hr2[:, g * ST:(g + 1) * ST], pss[g][:mm, :],
                            biases[1][m][:mm, :], 0.0, op0=OP.add, op1=OP.max)

            # ---- L3 (fp8 DR) -> h3 bf16 chunks ----
            cur = []
            for m, (m0, mm) in enumerate(HCH):
                pss = []
                for g in range(gs):
                    p = mm_psum.tile([128, ST], f32, tag="mmp")
                    pss.append(p)
                for g in range(gs):
                    nc.tensor.matmul(pss[g][:mm, :], w3d[:, :, m0:m0 + mm],
                                     hd2[:, :, g * ST:(g + 1) * ST],
                                     start=True, stop=False, perf_mode=DRM)
                for g in range(gs):
                    nc.tensor.matmul(pss[g][:mm, :], w3r[:, m0:m0 + mm],
                                     hr2[:, g * ST:(g + 1) * ST],
                                     start=False, stop=True)
                ht = hpool.tile([128, BST * ST], bf16, tag=f"h2_{m}")
                for g in range(gs):
                    nc.scalar.activation(ht[:mm, g * ST:(g + 1) * ST],
                                         pss[g][:mm, :], AF.Relu,
                                         bias=biases[2][m][:mm, :])
                cur.append(ht)
            return cur

        # ---- edge phase ----
        acc_state = {}

        def build_sts(blk):
            sts = []
            for i in range(16):
                cidx = blk * 16 + i
                st = spool.tile([128, NW], bf16, tag="st")
                nc.vector.tensor_scalar(
                    st[:, :], iota[:, :], dstloc[:, cidx:cidx + 1], None,
                    op0=OP.is_equal)
                sts.append(st)
            return sts

        def emit_scatter(blk, msgts, sts):
            for i, mt in enumerate(msgts):
                cidx = blk * 16 + i
                w = cidx // C
                cw = cidx % C
                if cw == 0:
                    at = acc_psum.tile([128, NW], f32, tag="acc")
                    acc_state["t"] = at
                nc.tensor.matmul(acc_state["t"][:, :], mt[:, :], sts[i][:, :],
                                 start=(cw == 0), stop=(cw == C - 1),
                                 skip_group_check=True)
                if cw == C - 1:
                    nc.scalar.activation(
                        aggrT[:, w * NW:(w + 1) * NW], acc_state["t"][:, :],
                        AF.Copy)

        prev_msgts = None
        prev_sts = None
        prev_blk = None
        for blk in range(NBLK):
            ind, inr = pending_in
            if blk + 1 < NBLK:
                pending_in = load_block(blk + 1)
            if prev_msgts is not None:
                prev_sts = build_sts(prev_blk)

            l1 = ("dr", wdr["mW1"], wrem["mW1"],
                  lambda g, t=ind: t[:, :, g * ST:(g + 1) * ST],
                  lambda g, t=inr: t[:, g * ST:(g + 1) * ST])

            h3 = mlp_front(l1, wdr["mW2"], wrem["mW2"],
                           wdr["mW3"], wrem["mW3"], mb, BST)

            # L4 weight-stationary: out [MSGD, ST] feature-major
            psl4 = []
            for g in range(BST):
                p4 = mm_psum.tile([128, ST], f32, tag="mmp")
                psl4.append(p4)
            for k, (k0, kk) in enumerate(HCH):
                for g in range(BST):
                    nc.tensor.matmul(psl4[g][:, :], mW4[k][:kk, :],
                                     h3[k][:kk, g * ST:(g + 1) * ST],
                                     start=(k == 0), stop=(k == 2))
            msgts = []
            for g in range(BST):
                mT = mtpool.tile([128, ST], bf16, tag=f"msgT{g}")
                nc.scalar.activation(mT[:, :], psl4[g][:, :], AF.Identity,
                                     bias=mb4c[:, :])
                mt4 = mtpool.tile([128, 4, 128], bf16, tag=f"msgt{g}")
                nc.sync.dma_start_transpose(mt4[:, :, :], mT[:, :])
                for e in range(4):
                    msgts.append(mt4[:, e, :])

            if prev_msgts is not None:
                emit_scatter(prev_blk, prev_msgts, prev_sts)
            prev_msgts, prev_blk = msgts, blk
        prev_sts = build_sts(prev_blk)
        emit_scatter(prev_blk, prev_msgts, prev_sts)

        # ---- node phase ----
        # L4 weight-stationary into feature-major [NF, nodes] PSUM (bias
        # fused in the copy), DMA-transposed to node-major chunks for the
        # pooling matmuls, which are deferred by one block.
        pool_acc = acc_psum.tile([128, NW], f32, tag="acc")

        def flush_pool(t0, gs, nts):
            for g in range(gs):
                for e in range(4):
                    tch = (t0 + g) * 4 + e
                    nc.tensor.matmul(pool_acc[:G, :NF],
                                     pmat[:, tch * G:(tch + 1) * G],
                                     nts[g][:, e, :],
                                     start=(tch == 0), stop=(tch == NCHK - 1),
                                     skip_group_check=True)

        pend_nt = None
        for t0 in range(0, NT, BST):
            gs = min(BST, NT - t0)

            nksrc = ("bf16", [
                ((lambda g, s=xT, t0=t0:
                  s[:, (t0 + g) * ST:(t0 + g + 1) * ST]), NF, nW1[0]),
                ((lambda g, s=aggrT, t0=t0:
                  s[:, (t0 + g) * ST:(t0 + g + 1) * ST]), MSGD, nW1[1])])

            h3n = mlp_front(nksrc, wdr["nW2"], wrem["nW2"],
                            wdr["nW3"], wrem["nW3"], nb, gs)

            psn = []
            for g in range(gs):
                p4 = mm_psum.tile([128, ST], f32, tag="mmp")
                psn.append(p4)
            for k, (k0, kk) in enumerate(HCH):
                for g in range(gs):
                    nc.tensor.matmul(psn[g][:, :], nW4[k][:kk, :],
                                     h3n[k][:kk, g * ST:(g + 1) * ST],
                                     start=(k == 0), stop=(k == 2))
            nts = []
            for g in range(gs):
                nT = mtpool.tile([128, ST], bf16, tag=f"msgT{g}")
                nc.scalar.activation(nT[:, :], psn[g][:, :], AF.Identity,
                                     bias=nb4c[:, :])
                nt4 = mtpool.tile([128, 4, 128], bf16, tag=f"msgt{g}")
                nc.sync.dma_start_transpose(nt4[:, :, :], nT[:, :])
                nts.append(nt4)
            if pend_nt is not None:
                flush_pool(*pend_nt)
            pend_nt = (t0, gs, nts)
        flush_pool(*pend_nt)

        pooled = apool.tile([G, NF], f32, tag="pooled")
        nc.vector.tensor_copy(pooled[:, :], pool_acc[:G, :NF])
        nc.sync.dma_start(d_out[:, :], pooled[:, :])

    nc.compile()
    return nc


def _prep_inputs(x, edge_index, edge_attr, batch, weights, C):
    """Host-side shard/gather/pad. Returns per-core in_maps."""
    E_pad = W * C * 128
    src = np.asarray(edge_index[0], np.int64)
    dst = np.asarray(edge_index[1], np.int64)

    order = np.argsort(dst, kind="stable")
    dsts = dst[order]
    srcs = src[order]

    xT = np.ascontiguousarray(np.asarray(x, np.float32).astype(BF16).T)
    eaT = np.ascontiguousarray(np.asarray(edge_attr, np.float32).astype(BF16).T)
    batch = np.asarray(batch, np.int64)

    bounds = np.searchsorted(dsts, np.arange(0, N_NODES + 1, NPC))

    def pad2(a, r, c):
        out = np.zeros((r, c), a.dtype)
        out[:a.shape[0], :a.shape[1]] = a
        return out

    FP8 = ml_dtypes.float8_e4m3fn
    wcommon = {}
    wcommon["mW4"] = pad2(weights["mW4"].astype(BF16), HIDP, MSGD)
    wcommon["nW1"] = pad2(weights["nW1"].astype(BF16), NF + MSGD, HIDP)
    wcommon["nW4"] = pad2(weights["nW4"].astype(BF16), HIDP, NF)
    for nm in ("mW1", "mW2", "mW3", "nW2", "nW3"):
        kr = KINP if nm == "mW1" else HIDP
        wp = pad2(weights[nm].astype(np.float32), kr, HIDP)
        dr = wp[:256].reshape(2, 128, HIDP).transpose(1, 0, 2)
        wcommon[f"{nm}dr"] = np.ascontiguousarray(
            dr.reshape(128, 2 * HIDP).astype(FP8))
        wcommon[f"{nm}rem"] = np.ascontiguousarray(wp[256:kr].astype(BF16))
    for i in range(1, 4):
        wcommon[f"mb{i}"] = pad2(
            weights[f"mb{i}"].reshape(HID, 1).astype(np.float32), HIDP, 1)
        wcommon[f"nb{i}"] = pad2(
            weights[f"nb{i}"].reshape(HID, 1).astype(np.float32), HIDP, 1)
    wcommon["mb4c"] = np.ascontiguousarray(
        weights["mb4"].reshape(MSGD, 1).astype(np.float32))
    wcommon["nb4c"] = np.ascontiguousarray(
        weights["nb4"].reshape(NF, 1).astype(np.float32))

    garange = np.arange(G)
    in_maps = []
    for k in range(NCORES):
        sl = slice(int(bounds[k]), int(bounds[k + 1]))
        eidx = order[sl]
        dloc = dsts[sl] - k * NPC
        srck = srcs[sl]
        win = dloc // NW
        cnt = np.bincount(win, minlength=W)

        starts = np.repeat(np.arange(W) * C * 128, cnt)
        within = np.arange(len(dloc)) - np.repeat(np.cumsum(cnt) - cnt, cnt)
        pos = starts + within

        min_dr = np.zeros((128, 2, E_pad), FP8)
        min_dr[:, 0, pos] = xT[:, k * NPC + dloc].astype(FP8)
        min_dr[:, 1, pos] = xT[:, srck].astype(FP8)
        min_rem = np.zeros((128, E_pad), BF16)
        min_rem[:EF, pos] = eaT[:, eidx]

        dl = np.full(E_pad, -1.0, np.float32)
        dl[pos] = (dloc - win * NW).astype(np.float32)
        dstloc = np.ascontiguousarray(dl.reshape(E_pad // 128, 128).T)

        xTn = np.zeros((NF, NP2), BF16)
        xTn[:, :NPC] = xT[:, k * NPC:(k + 1) * NPC]

        bl = np.full(NP2, -1, np.int64)
        bl[:NPC] = batch[k * NPC:(k + 1) * NPC]
        P = (bl[:, None] == garange[None, :]).astype(BF16)
        pmat = np.ascontiguousarray(
            P.reshape(NCHK, 128, G).transpose(1, 0, 2).reshape(128, NCHK * G))

        in_map = dict(wcommon)
        in_map.update(min_dr=min_dr, min_rem=min_rem, dstloc=dstloc, xT=xTn, pmat=pmat)
        in_maps.append(in_map)
    return in_maps


def kernel(**inputs):
    global LAST_EXEC_NS
    from concourse.bass_utils import run_bass_kernel_spmd

    x = np.asarray(inputs["x"], np.float32)
    edge_index = np.asarray(inputs["edge_index"])
    edge_attr = np.asarray(inputs["edge_attr"], np.float32)
    batch = np.asarray(inputs["batch"])

    # chunk count per window from the actual data (uniform across cores)
    dst = np.asarray(edge_index[1], np.int64)
    dloc_all = dst % NPC
    core_all = dst // NPC
    win_all = dloc_all // NW
    cnt = np.bincount(core_all * W + win_all, minlength=NCORES * W)
    C = int(np.ceil(cnt.max() / 128.0))
    C = max(C, 8)
    while (W * C) % 16 != 0:
        C += 1

    key = C
    if key not in _BUILD_CACHE:
        _BUILD_CACHE[key] = _build_nc(C)
    nc = _BUILD_CACHE[key]

    in_maps = _prep_inputs(x, edge_index, edge_attr, batch, inputs, C)

    res = run_bass_kernel_spmd(nc, in_maps, list(range(NCORES)), trace=TRACE)
    LAST_EXEC_NS = res.exec_time_ns

    total = np.zeros((G, NF), np.float64)
    for r in res.results:
        total += np.asarray(r["partial"], np.float64)

    counts = np.bincount(np.asarray(batch, np.int64), minlength=G)
    pooled = (total / np.maximum(counts, 1)[:, None]).astype(np.float32)
    out = pooled @ np.asarray(inputs["linW"], np.float32) + np.asarray(
        inputs["linb"], np.float32)
    return out.astype(np.float32)
